# revision 1
# baseline (speedup 1.0000x reference)
"""Trainium2 Bass kernel for nn_DecoderLayer_84404697301735.

3-sublayer decoder (self-attn w/ char rel-pos, cross-attn to char encoder
w/ rel-pos, cross-attn to word encoder w/ word-level pos) + FFN.

Sharding: 8 cores = 4 batch x 2 interleaved query-tile halves.  Each core
computes 512 query rows end-to-end (feature-major layout); K/V projections
over the full 1024 keys are duplicated within a batch pair.  No collectives.

Relative-position logits: Band[i,t] = Q[i] . E[t] via matmul against a
host-built extended pos table E (clip folded in, causal mask folded into a
65th ones-row of Q), then a skewed-stride DMA read from a DRAM round-trip
converts band (query-relative) layout to absolute key layout.  The per-core
query-offset parity is folded into E so the program is core-independent.
"""

import numpy as np
import ml_dtypes

import concourse.bass as bass
import concourse.tile as tile
from concourse import bacc, mybir
from concourse.bass_utils import run_bass_kernel_spmd

BF16 = ml_dtypes.bfloat16
F32 = mybir.dt.float32
F32R = mybir.dt.float32r
BF = mybir.dt.bfloat16

D = 1024
H = 16
DH = 64
S_OWN = 512          # own query rows per core
LK = 1024            # keys
DFF = 4096
M = 128              # pos clip radius
NPOS = 2 * M + 1     # 257
WE = 1152            # extended pos table width
SCALE = float(DH) ** 0.5   # 8.0
EPS = 1e-5
NEG = -1e30

AL = mybir.AluOpType
AF = mybir.ActivationFunctionType

# bias_cat column layout (each unit = 1 col of [128, n] per-partition stripes)
_BIAS_SECTS = [
    ("qkv", 24), ("o1", 8), ("q2", 8), ("k2", 8), ("v2", 8), ("o2", 8),
    ("q3", 8), ("k3", 8), ("v3", 8), ("o3", 8), ("f1", 32), ("f2", 8),
    ("ln1g", 8), ("ln1b", 8), ("ln2g", 8), ("ln2b", 8), ("ln3g", 8), ("ln3b", 8),
]
BIAS_COL = {}
_c = 0
for _n, _w in _BIAS_SECTS:
    BIAS_COL[_n] = _c
    _c += _w
NBIAS = _c  # 184


def _t0(l, k):
    """Static skew-read base column for query tile l, key tile k."""
    return min(max(128 * (k - 2 * l) + 512, 128), 1024)


def _skew_segments(l, c):
    """Plan skew DMAs for strip (l, chunk c): list of (dest_col, n_ktiles,
    t0, affine) where affine=True means source cols advance with dest."""
    segs = []
    ks = list(range(4 * c, 4 * c + 4))
    i = 0
    while i < len(ks):
        k = ks[i]
        raw = 128 * (k - 2 * l) + 512
        t0 = _t0(l, k)
        clamped = raw != t0
        j = i + 1
        if not clamped:
            # extend affine run over consecutive unclamped ks
            while j < len(ks):
                k2 = ks[j]
                raw2 = 128 * (k2 - 2 * l) + 512
                if raw2 != _t0(l, k2):
                    break
                j += 1
            segs.append(((ks[i] - 4 * c) * 128, j - i, t0, True))
        else:
            # extend run over consecutive clamped ks with same t0
            while j < len(ks):
                k2 = ks[j]
                raw2 = 128 * (k2 - 2 * l) + 512
                if raw2 == _t0(l, k2) or _t0(l, k2) != t0:
                    break
                j += 1
            segs.append(((ks[i] - 4 * c) * 128, j - i, t0, False))
        i = j
    return segs


def _emit(nc, tc, ctx, T, debug=False):
    """Emit the whole per-core program.  T: dict name -> dram AP."""
    te, ve, sc, gp, sy = nc.tensor, nc.vector, nc.scalar, nc.gpsimd, nc.sync

    singles = ctx.enter_context(tc.tile_pool(name="singles", bufs=1))
    psum = ctx.enter_context(tc.tile_pool(name="psum", bufs=1, space="PSUM"))
    wpool = ctx.enter_context(tc.tile_pool(name="wpool", bufs=8))
    work = ctx.enter_context(tc.tile_pool(name="work", bufs=3))
    ptpool = ctx.enter_context(tc.tile_pool(name="ptpool", bufs=4))
    ppool = ctx.enter_context(tc.tile_pool(name="ppool", bufs=4))
    dram = ctx.enter_context(tc.tile_pool(name="dramp", bufs=4, space="DRAM"))
    smalls = ctx.enter_context(tc.tile_pool(name="smalls", bufs=4))
    # PSUM budget (8 banks): mm=4, s=2, pv=2

    # ---- persistent SBUF ----
    bias_sb = singles.tile([128, NBIAS], F32)
    sy.dma_start(bias_sb, T["bias"])
    e1t_sb = singles.tile([65, WE], BF)
    sy.dma_start(e1t_sb, T["e1t"])
    e2t_sb = singles.tile([65, WE], BF)
    sy.dma_start(e2t_sb, T["e2t"])
    g3t_sb = singles.tile([128, LK], BF)
    sy.dma_start(g3t_sb, T["g3t"])
    eps_sb = singles.tile([1, 1], F32)
    ve.memset(eps_sb, EPS)
    ones_sb = singles.tile([128, 1], BF)
    ve.memset(ones_sb, 1.0)

    xres = singles.tile([128, 8, S_OWN], F32)      # residual stream (feature-major)
    sy.dma_start(xres, T["xow"].rearrange("(a p) r -> p a r", p=128))
    xbf = singles.tile([128, 8, S_OWN], BF)        # bf16 copy for proj rhs
    sy.dma_start(xbf, T["xob"].rearrange("(a p) r -> p a r", p=128))
    enc = singles.tile([128, 8, LK], BF)           # current sublayer's enc input
    kt_all = singles.tile([64, H, LK], BF)         # K^T per head
    v_all = singles.tile([128, H, 8, 65], BF)      # V key-major + ones col
    gp.memset(v_all, 1.0)
    qt = [singles.tile([65, S_OWN], BF, name=f"qt{h}", tag=f"qt{h}") for h in range(H)]
    for h in range(H):
        gp.memset(qt[h][64:65, :], 1.0)
    aT = singles.tile([128, 8, S_OWN], BF)         # attention output (feature-major)
    h1 = singles.tile([128, 32, S_OWN], BF)        # FFN hidden
    ident64 = singles.tile([64, 64], BF)
    from concourse.masks import make_identity
    make_identity(nc, ident64)

    def load_enc(name):
        sy.dma_start(enc, T[name].rearrange("(a p) r -> p a r", p=128))

    def bias_ap(col, base=0, size=128):
        return bias_sb[base:base + size, col:col + 1]

    # ---------------- projections ----------------
    def proj(wname, ct0, nct, bias0, rhs_fn, nrc, nkt, epilogue):
        wd = T[wname]
        for ct in range(nct):
            pss = []
            for rc in range(nrc):
                ps = psum.tile([128, 512], F32, tag="mm", bufs=4, name=f"ps_{wname}_{ct}_{rc}")
                pss.append(ps)
            for kt in range(nkt):
                wt = wpool.tile([128, 128], BF, tag="w", name=f"wt_{wname}_{ct}_{kt}")
                sy.dma_start(wt, wd[kt, ct0 + ct])
                for rc in range(nrc):
                    te.matmul(pss[rc], lhsT=wt, rhs=rhs_fn(kt, rc),
                              start=(kt == 0), stop=(kt == nkt - 1))
            for rc in range(nrc):
                epilogue(ct, rc, pss[rc])

    def rhs_own(kt, rc):
        return xbf[:, kt, :]

    def rhs_enc(kt, rc):
        return enc[:, kt, 512 * rc:512 * rc + 512]

    def q_ep(bias0):
        def ep(ct, rc, ps):
            for sub in range(2):
                h = 2 * ct + sub
                ve.tensor_scalar_add(qt[h][0:64, :], ps[64 * sub:64 * sub + 64, :],
                                     bias_ap(bias0 + ct, 64 * sub, 64))
        return ep

    def k_ep(bias0, add_g):
        def ep(ct, rc, ps):
            for sub in range(2):
                h = 2 * ct + sub
                dst = kt_all[0:64, h, 512 * rc:512 * rc + 512]
                src = ps[64 * sub:64 * sub + 64, :]
                b = bias_ap(bias0 + ct, 64 * sub, 64)
                if add_g:
                    ve.scalar_tensor_tensor(
                        dst, src, b, g3t_sb[64 * sub:64 * sub + 64, 512 * rc:512 * rc + 512],
                        op0=AL.add, op1=AL.add)
                else:
                    ve.tensor_scalar_add(dst, src, b)
        return ep

    def v_ep(bias0):
        def ep(ct, rc, ps):
            for sub in range(2):
                h = 2 * ct + sub
                vt = work.tile([64, 512], BF, tag="vt", name=f"vt_{h}_{rc}")
                ve.tensor_scalar_add(vt, ps[64 * sub:64 * sub + 64, :],
                                     bias_ap(bias0 + ct, 64 * sub, 64))
                for j in range(4):
                    tp = psum.tile([128, 64], BF, tag="s", bufs=2,
                                   name=f"vtp_{h}_{rc}_{j}")
                    te.transpose(tp, vt[:, 128 * j:128 * j + 128], ident64)
                    ve.tensor_copy(v_all[:, h, 4 * rc + j, 0:64], tp)
        return ep

    # ---------------- attention ----------------
    def attention(sl):
        e_sb = e1t_sb if sl == 1 else e2t_sb
        for h in range(H):
            for l in range(4):
                if sl == 1:
                    nchunks = 1 if l < 2 else 2
                else:
                    nchunks = 2
                nkt = 4 * nchunks
                # --- band: Q x E (+ ones x maskrow) -> bf16 -> DRAM ---
                bd = None
                if sl != 3:
                    bsb = work.tile([128, WE], BF, tag="bsb", bufs=2, name=f"bsb{sl}_{h}_{l}")
                    for seg in range(3):
                        bp = psum.tile([128, 384], F32, tag="mm", bufs=4,
                                       name=f"bp{sl}_{h}_{l}_{seg}")
                        te.matmul(bp, lhsT=qt[h][:, 128 * l:128 * l + 128],
                                  rhs=e_sb[:, 384 * seg:384 * seg + 384],
                                  start=True, stop=True)
                        ve.tensor_copy(bsb[:, 384 * seg:384 * seg + 384], bp)
                    bd = dram.tile([128, WE], BF, tag="bd", name=f"bd{sl}_{h}_{l}")
                    sy.dma_start(bd, bsb)
                    if debug and sl == 1 and h == 0 and l == 0:
                        sy.dma_start(T["d_band"], bsb)
                # --- scores + softmax (no max-sub; logits are small) ---
                p_tiles = []
                for c in range(nchunks):
                    sps = psum.tile([128, 512], F32, tag="s", bufs=2, name=f"s{sl}_{h}_{l}_{c}")
                    te.matmul(sps, lhsT=qt[h][0:64, 128 * l:128 * l + 128],
                              rhs=kt_all[0:64, h, 512 * c:512 * c + 512],
                              start=True, stop=True)
                    pt = ppool.tile([128, 512], BF, tag="p", name=f"p{sl}_{h}_{l}_{c}")
                    if sl == 3:
                        sc.activation(pt, sps, AF.Exp, scale=1.0 / SCALE)
                    else:
                        strip = work.tile([128, 512], BF, tag="strip",
                                          name=f"strip{sl}_{h}_{l}_{c}")
                        for (dcol, n, t0, affine) in _skew_segments(l, c):
                            if affine:
                                src = bass.AP(tensor=bd.tensor, offset=bd.offset + t0,
                                              ap=[[WE - 1, 128], [1, 128 * n]])
                                sy.dma_start(strip[:, dcol:dcol + 128 * n], src)
                            else:
                                src = bass.AP(tensor=bd.tensor, offset=bd.offset + t0,
                                              ap=[[WE - 1, 128], [0, n], [1, 128]])
                                dst = strip[:, dcol:dcol + 128 * n].rearrange(
                                    "p (n w) -> p n w", w=128)
                                sy.dma_start(dst, src)
                        lg = work.tile([128, 512], F32, tag="lg", bufs=4,
                                       name=f"lg{sl}_{h}_{l}_{c}")
                        ve.scalar_tensor_tensor(lg, sps, 1.0 / SCALE, strip,
                                                op0=AL.mult, op1=AL.add)
                        sc.activation(pt, lg, AF.Exp)
                        if debug and sl == 1 and h == 0 and l == 0 and c == 0:
                            sy.dma_start(T["d_strip"], strip)
                            sy.dma_start(T["d_lg"], lg)
                            sy.dma_start(T["d_p"], pt)
                    p_tiles.append(pt)
                # --- P^T via DMA transpose; PV accumulate; normalize ---
                pv = psum.tile([65, 128], F32, tag="pv", bufs=2, name=f"pv{sl}_{h}_{l}")
                for kt in range(nkt):
                    ptt = ptpool.tile([128, 128], BF, tag="pt",
                                      name=f"ptt{sl}_{h}_{l}_{kt}")
                    sc.dma_start_transpose(
                        ptt, p_tiles[kt // 4][:, 128 * (kt % 4):128 * (kt % 4) + 128])
                    te.matmul(pv, lhsT=v_all[:, h, kt, :], rhs=ptt,
                              start=(kt == 0), stop=(kt == nkt - 1))
                rz = smalls.tile([1, 128], F32, tag="rz", name=f"rz{sl}_{h}_{l}")
                ve.reciprocal(rz, pv[64:65, :])
                zb = smalls.tile([64, 128], F32, tag="zb", name=f"zb{sl}_{h}_{l}")
                gp.partition_broadcast(zb, rz)
                ve.tensor_mul(aT[64 * (h % 2):64 * (h % 2) + 64, h // 2,
                                 128 * l:128 * l + 128],
                              pv[0:64, :], zb)

    # ---------------- output proj + residual ----------------
    def o_proj(wname, bias0):
        def ep(ct, rc, ps):
            ve.scalar_tensor_tensor(xres[:, ct, :], ps, bias_ap(bias0 + ct),
                                    xres[:, ct, :], op0=AL.add, op1=AL.add)
        proj(wname, 0, 8, bias0, lambda kt, rc: aT[:, kt, :], 1, 8, ep)

    # ---------------- layernorm (feature-major) ----------------
    def layer_norm(gname, bname, final_out=None):
        s1 = psum.tile([1, 512], F32, tag="pv", bufs=2, name=f"lns1_{gname}_{1 if final_out is None else 2}")
        s2 = psum.tile([1, 512], F32, tag="pv", bufs=2, name=f"lns2_{gname}_{1 if final_out is None else 2}")
        for dt in range(8):
            bx = work.tile([128, 512], BF, tag="lnbx", bufs=3, name=f"lnbx_{gname}_{dt}")
            gp.tensor_copy(bx, xres[:, dt, :])
            sq = work.tile([128, 512], BF, tag="lnsq", bufs=3, name=f"lnsq_{gname}_{dt}")
            ve.tensor_mul(sq, xres[:, dt, :], xres[:, dt, :])
            te.matmul(s1, lhsT=ones_sb, rhs=bx,
                      start=(dt == 0), stop=(dt == 7))
            te.matmul(s2, lhsT=ones_sb, rhs=sq,
                      start=(dt == 0), stop=(dt == 7))
        mean = smalls.tile([1, 512], F32, tag="ln", name=f"lnmean_{gname}")
        ve.tensor_scalar_mul(mean, s1, 1.0 / D)
        rstd = smalls.tile([1, 512], F32, tag="ln", name=f"lnrstd_{gname}")
        ve.tensor_mul(rstd, mean, mean)                                  # mean^2
        ve.scalar_tensor_tensor(rstd, s2, 1.0 / D, rstd,
                                op0=AL.mult, op1=AL.subtract)            # var
        sc.activation(rstd, rstd, AF.Sqrt, bias=eps_sb)                  # sd
        ve.reciprocal(rstd, rstd)                                        # 1/sd
        mb = work.tile([128, 512], F32, tag="lnb", bufs=2, name=f"lnmb_{gname}")
        gp.partition_broadcast(mb, mean)
        rb = work.tile([128, 512], F32, tag="lnb", bufs=2, name=f"lnrb_{gname}")
        gp.partition_broadcast(rb, rstd)
        gcol, bcol = BIAS_COL[gname], BIAS_COL[bname]
        for dt in range(8):
            mgr = work.tile([128, 512], F32, tag="lg", bufs=4, name=f"lnmgr_{gname}_{dt}")
            ve.scalar_tensor_tensor(mgr, mb, bias_ap(gcol + dt), rb,
                                    op0=AL.mult, op1=AL.mult)
            cc = work.tile([128, 512], F32, tag="lg", bufs=4, name=f"lncc_{gname}_{dt}")
            ve.tensor_scalar(cc, mgr, -1.0, bias_ap(bcol + dt),
                             op0=AL.mult, op1=AL.add)
            t = work.tile([128, 512], F32, tag="lg", bufs=4, name=f"lnt_{gname}_{dt}")
            ve.scalar_tensor_tensor(t, xres[:, dt, :], bias_ap(gcol + dt), rb,
                                    op0=AL.mult, op1=AL.mult)
            if final_out is not None:
                ot = work.tile([128, 512], F32, tag="lg", bufs=4, name=f"lnot_{gname}_{dt}")
                ve.tensor_add(ot, t, cc)
                sy.dma_start(final_out[128 * dt:128 * dt + 128, :], ot)
            else:
                ve.tensor_add(xres[:, dt, :], t, cc)
                sc.activation(xbf[:, dt, :], xres[:, dt, :], AF.Copy)

    # ================= sublayer 1 =================
    load_enc("sfb")
    proj("wqkv", 0, 8, BIAS_COL["qkv"], rhs_own, 1, 8, q_ep(BIAS_COL["qkv"]))
    proj("wqkv", 8, 8, BIAS_COL["qkv"] + 8, rhs_enc, 2, 8,
         k_ep(BIAS_COL["qkv"] + 8, False))
    proj("wqkv", 16, 8, BIAS_COL["qkv"] + 16, rhs_enc, 2, 8,
         v_ep(BIAS_COL["qkv"] + 16))
    if debug:
        sy.dma_start(T["d_qt"], qt[0])
        sy.dma_start(T["d_kt"], kt_all[0:64, 0, :])
        sy.dma_start(T["d_v"], v_all[:, 0, :, :])
    attention(1)
    if debug:
        sy.dma_start(T["d_at"], aT)
    o_proj("wo1", BIAS_COL["o1"])
    layer_norm("ln1g", "ln1b")
    if debug:
        sy.dma_start(T["d_x1"], xres)

    # ================= sublayer 2 =================
    load_enc("chb")
    proj("wq2", 0, 8, BIAS_COL["q2"], rhs_own, 1, 8, q_ep(BIAS_COL["q2"]))
    proj("wk2", 0, 8, BIAS_COL["k2"], rhs_enc, 2, 8, k_ep(BIAS_COL["k2"], False))
    proj("wv2", 0, 8, BIAS_COL["v2"], rhs_enc, 2, 8, v_ep(BIAS_COL["v2"]))
    attention(2)
    o_proj("wo2", BIAS_COL["o2"])
    layer_norm("ln2g", "ln2b")

    # ================= sublayer 3 =================
    load_enc("wdb")
    proj("wq3", 0, 8, BIAS_COL["q3"], rhs_own, 1, 8, q_ep(BIAS_COL["q3"]))
    proj("wk3", 0, 8, BIAS_COL["k3"], rhs_enc, 2, 8, k_ep(BIAS_COL["k3"], True))
    proj("wv3", 0, 8, BIAS_COL["v3"], rhs_enc, 2, 8, v_ep(BIAS_COL["v3"]))
    attention(3)
    o_proj("wo3", BIAS_COL["o3"])
    layer_norm("ln3g", "ln3b")

    # ================= FFN =================
    def f1_ep(ct, rc, ps):
        sc.activation(h1[:, ct, :], ps, AF.Relu, bias=bias_ap(BIAS_COL["f1"] + ct))
    proj("wf1", 0, 32, BIAS_COL["f1"], rhs_own, 1, 8, f1_ep)

    def f2_ep(ct, rc, ps):
        ve.scalar_tensor_tensor(xres[:, ct, :], ps, bias_ap(BIAS_COL["f2"] + ct),
                                xres[:, ct, :], op0=AL.add, op1=AL.add)
    proj("wf2", 0, 8, BIAS_COL["f2"], lambda kt, rc: h1[:, kt, :], 1, 32, f2_ep)

    layer_norm("ln3g", "ln3b", final_out=T["yT"])


def build_nc(debug=False):
    nc = bacc.Bacc("TRN2", target_bir_lowering=False, debug=False)
    T = {}

    def din(name, shape, dt=BF):
        T[name] = nc.dram_tensor(name, shape, dt, kind="ExternalInput").ap()

    din("xow", [D, S_OWN], F32)
    din("xob", [D, S_OWN])
    din("sfb", [D, LK])
    din("chb", [D, LK])
    din("wdb", [D, LK])
    din("wqkv", [8, 24, 128, 128])
    for w in ["wo1", "wq2", "wk2", "wv2", "wo2", "wq3", "wk3", "wv3", "wo3"]:
        din(w, [8, 8, 128, 128])
    din("wf1", [8, 32, 128, 128])
    din("wf2", [32, 8, 128, 128])
    din("bias", [128, NBIAS], F32)
    din("e1t", [65, WE])
    din("e2t", [65, WE])
    din("g3t", [128, LK])
    T["yT"] = nc.dram_tensor("yT", [D, S_OWN], F32, kind="ExternalOutput").ap()
    if debug:
        def dout(name, shape, dt=BF):
            T[name] = nc.dram_tensor(name, shape, dt, kind="ExternalOutput").ap()
        dout("d_qt", [65, S_OWN])
        dout("d_kt", [64, LK])
        dout("d_v", [128, 8, 65])
        dout("d_at", [128, 8, S_OWN])
        dout("d_x1", [128, 8, S_OWN], F32)
        dout("d_band", [128, WE])
        dout("d_strip", [128, 512])
        dout("d_lg", [128, 512], F32)
        dout("d_p", [128, 512])

    from contextlib import ExitStack
    with tile.TileContext(nc) as tc:
        with ExitStack() as ctx:
            _emit(nc, tc, ctx, T, debug=debug)
    nc.compile()
    return nc


_NC = None


def _get_nc():
    global _NC
    if _NC is None:
        _NC = build_nc()
    return _NC


# ======================= host side =======================

def _own_rows(pi):
    return np.concatenate([np.arange(128 * (2 * l + pi), 128 * (2 * l + pi) + 128)
                           for l in range(4)])


def _tile_w(w):
    K, N = w.shape
    return np.ascontiguousarray(
        w.reshape(K // 128, 128, N // 128, 128).transpose(0, 2, 1, 3)
    ).astype(BF16)


def _stripe(v):
    """bias vector [n*128] -> [128, n] per-partition stripes (fp32)."""
    n = v.shape[0] // 128
    return np.ascontiguousarray(v.reshape(n, 128).T).astype(np.float32)


def _build_E(pos_scaled, pi, causal):
    t = np.arange(WE)
    r = t - 512 - 128 * pi
    idx = np.clip(r, -M, M) + M
    e = np.zeros((65, WE), np.float32)
    e[0:64, :] = pos_scaled[idx].T
    if causal:
        e[64, :] = np.where(r > 0, NEG, 0.0)
    return e.astype(BF16)


def _qpos(sentence_lengths):
    s = np.asarray(sentence_lengths, np.int64)
    offsets = s - np.cumsum(s)
    B = int(s.sum())
    return np.repeat(offsets, s)[:B] + np.arange(B)


def _host_prep(inp):
    qkv_w = np.asarray(inp["qkv_w"], np.float32)
    wq = qkv_w.reshape(D, H, 3, DH)
    wqkv_r = np.concatenate([wq[:, :, 0], wq[:, :, 1], wq[:, :, 2]], axis=1)
    wqkv_r = wqkv_r.reshape(D, 3 * D)
    qb = np.asarray(inp["qkv_b"], np.float32).reshape(H, 3, DH)
    qkv_b_r = np.concatenate([qb[:, 0], qb[:, 1], qb[:, 2]], axis=0).reshape(3 * D)

    bias = np.zeros((128, NBIAS), np.float32)

    def put(name, vec):
        c = BIAS_COL[name]
        s = _stripe(np.asarray(vec, np.float32))
        bias[:, c:c + s.shape[1]] = s

    put("qkv", qkv_b_r)
    for n, k in [("o1", "o1_b"), ("q2", "q2_b"), ("k2", "k2_b"), ("v2", "v2_b"),
                 ("o2", "o2_b"), ("q3", "q3_b"), ("k3", "k3_b"), ("v3", "v3_b"),
                 ("o3", "o3_b"), ("f1", "f1_b"), ("f2", "f2_b"),
                 ("ln1g", "ln1_g"), ("ln1b", "ln1_b"), ("ln2g", "ln2_g"),
                 ("ln2b", "ln2_b"), ("ln3g", "ln3_g"), ("ln3b", "ln3_b")]:
        put(n, inp[k])

    weights = {
        "wqkv": _tile_w(wqkv_r),
        "wo1": _tile_w(np.asarray(inp["o1_w"], np.float32)),
        "wq2": _tile_w(np.asarray(inp["q2_w"], np.float32)),
        "wk2": _tile_w(np.asarray(inp["k2_w"], np.float32)),
        "wv2": _tile_w(np.asarray(inp["v2_w"], np.float32)),
        "wo2": _tile_w(np.asarray(inp["o2_w"], np.float32)),
        "wq3": _tile_w(np.asarray(inp["q3_w"], np.float32)),
        "wk3": _tile_w(np.asarray(inp["k3_w"], np.float32)),
        "wv3": _tile_w(np.asarray(inp["v3_w"], np.float32)),
        "wo3": _tile_w(np.asarray(inp["o3_w"], np.float32)),
        "wf1": _tile_w(np.asarray(inp["f1_w"], np.float32)),
        "wf2": _tile_w(np.asarray(inp["f2_w"], np.float32)),
        "bias": bias,
    }

    pos1s = np.asarray(inp["pos1"], np.float32) / SCALE
    pos2s = np.asarray(inp["pos2"], np.float32) / SCALE
    pos3 = np.asarray(inp["pos3"], np.float32)
    e1 = [_build_E(pos1s, pi, True) for pi in range(2)]
    e2 = [_build_E(pos2s, pi, False) for pi in range(2)]

    qpos = _qpos(inp["sentence_lengths"])
    g3 = []
    for b in range(4):
        idx = np.clip(np.arange(LK) - int(qpos[b]), -M, M) + M
        g = pos3[idx].T.astype(BF16)          # [64, LK]
        g3.append(np.concatenate([g, g], axis=0))  # [128, LK] duplicated

    x = np.asarray(inp["self_input"], np.float32)
    ch = np.asarray(inp["char_enc"], np.float32)
    wd = np.asarray(inp["word_enc"], np.float32)

    in_maps = []
    for core in range(8):
        b, pi = core // 2, core % 2
        rows = _own_rows(pi)
        xT = np.ascontiguousarray(x[b].T)            # [D, 1024]
        m = dict(weights)
        m["xow"] = np.ascontiguousarray(xT[:, rows])
        m["xob"] = m["xow"].astype(BF16)
        m["sfb"] = xT.astype(BF16)
        m["chb"] = np.ascontiguousarray(ch[b].T).astype(BF16)
        m["wdb"] = np.ascontiguousarray(wd[b].T).astype(BF16)
        m["e1t"] = e1[pi]
        m["e2t"] = e2[pi]
        m["g3t"] = g3[b]
        in_maps.append(m)
    return in_maps


def _fast_path_ok(inp):
    lam = np.asarray(inp["look_ahead_mask"])
    B, Lq = 4, 1024
    if lam.shape != (1, 1, Lq, Lq):
        return False
    causal = np.triu(np.ones((Lq, Lq), bool), k=1)
    if not np.array_equal(lam[0, 0].astype(bool), causal):
        return False
    if np.asarray(inp["char_mask"]).any() or np.asarray(inp["word_mask"]).any():
        return False
    if np.asarray(inp["sentence_lengths"]).sum() != B:
        return False
    return True


def _numpy_reference(inp):
    """Pure-numpy fallback (slow but exact) for unexpected mask patterns."""
    f = lambda k: np.asarray(inp[k], np.float32)

    def ln(x, g, b):
        m = x.mean(-1, keepdims=True)
        v = ((x - m) ** 2).mean(-1, keepdims=True)
        return (x - m) / np.sqrt(v + EPS) * g + b

    def split_heads(x):
        B, S, _ = x.shape
        return x.reshape(B, S, H, DH).transpose(0, 2, 1, 3)

    def softmax(x):
        x = x - x.max(-1, keepdims=True)
        e = np.exp(x)
        return e / e.sum(-1, keepdims=True)

    def attn(Q, K, V, pl, mask):
        logits = (np.einsum('bhid,bhjd->bhij', Q, K) + pl) / SCALE
        logits = np.where(mask, -np.inf, logits)
        p = softmax(logits)
        out = np.einsum('bhij,bhjd->bhid', p, V)
        B, h, S, dh = out.shape
        return out.transpose(0, 2, 1, 3).reshape(B, S, h * dh)

    def char_pos(emb, lq, lk):
        idx = np.clip(np.arange(lk)[None, :] - np.arange(lq)[:, None], -M, M) + M
        return emb[idx]

    x0 = f("self_input")
    B, Lq, _ = x0.shape
    qkv = (x0 @ f("qkv_w") + f("qkv_b")).reshape(B, Lq, H, 3 * DH).transpose(0, 2, 1, 3)
    Q, K, V = np.split(qkv, 3, axis=-1)
    pl = np.einsum('bhid,ijd->bhij', Q, char_pos(f("pos1"), Lq, Lq))
    a = attn(Q, K, V, pl, np.asarray(inp["look_ahead_mask"])) @ f("o1_w") + f("o1_b")
    x = ln(a + x0, f("ln1_g"), f("ln1_b"))

    ce = f("char_enc")
    Q = split_heads(x @ f("q2_w") + f("q2_b"))
    K = split_heads(ce @ f("k2_w") + f("k2_b"))
    V = split_heads(ce @ f("v2_w") + f("v2_b"))
    pl = np.einsum('bhid,ijd->bhij', Q, char_pos(f("pos2"), Lq, ce.shape[1]))
    a = attn(Q, K, V, pl, np.asarray(inp["char_mask"])) @ f("o2_w") + f("o2_b")
    x = ln(a + x, f("ln2_g"), f("ln2_b"))

    we = f("word_enc")
    Q = split_heads(x @ f("q3_w") + f("q3_b"))
    K = split_heads(we @ f("k3_w") + f("k3_b"))
    V = split_heads(we @ f("v3_w") + f("v3_b"))
    qpos = _qpos(inp["sentence_lengths"])
    idx = np.clip(np.arange(we.shape[1])[None, :] - qpos[:, None], -M, M) + M
    pl = np.einsum('bhid,bjd->bhij', Q, f("pos3")[idx])
    a = attn(Q, K, V, pl, np.asarray(inp["word_mask"])) @ f("o3_w") + f("o3_b")
    x = ln(a + x, f("ln3_g"), f("ln3_b"))

    ffn = np.maximum(x @ f("f1_w") + f("f1_b"), 0.0) @ f("f2_w") + f("f2_b")
    return ln(ffn + x, f("ln3_g"), f("ln3_b"))


def kernel(**inputs) -> np.ndarray:
    if not _fast_path_ok(inputs):
        return _numpy_reference(inputs)
    nc = _get_nc()
    in_maps = _host_prep(inputs)
    res = run_bass_kernel_spmd(nc, in_maps, list(range(8)))
    y = np.empty((4, 1024, 1024), np.float32)
    for core in range(8):
        b, pi = core // 2, core % 2
        yT = res.results[core]["yT"]
        y[b, _own_rows(pi), :] = yT.T
    return y



# revision 4
# speedup vs baseline: 3.4952x; 3.4952x over previous
"""Trainium2 Bass kernel for nn_DecoderLayer_84404697301735 (v2).

3-sublayer decoder (self-attn w/ char rel-pos, cross-attn to char encoder
w/ rel-pos, cross-attn to word encoder w/ word-level pos) + FFN.

Sharding: 8 cores = 4 batch x 2 interleaved query-tile halves; each core
computes 512 query rows end-to-end; K/V duplicated within a batch pair.
No collectives.  Program is parity-uniform (SPMD); parity is folded into
the band-table column origin (host data).

Key design vs v1: scores are computed TRANSPOSED ([keys, queries]) so the
exp'd probabilities feed PV matmuls directly -- no P transposes.  Relative
position logits use softmax invariance (subtract Q.E_base per query) so the
band only touches +-1 diagonal key tiles; band strips are skew-read from a
DRAM round trip (query-major, DMA friendly) and transpose-accumulated into
the scores PSUM via identity matmuls.  V is computed key-major directly
(enc stationary).  K/V biases fold away analytically.
"""

import numpy as np
import ml_dtypes

import concourse.bass as bass
import concourse.tile as tile
from concourse import bacc, mybir
from concourse.bass_utils import run_bass_kernel_spmd
from concourse.masks import make_identity

BF16 = ml_dtypes.bfloat16
F32 = mybir.dt.float32
BF = mybir.dt.bfloat16

D = 1024
H = 16
DH = 64
S = 512              # own query rows per core
LK = 1024            # keys
M = 128              # pos clip radius
SCALE = 8.0
EPS = 1e-5
NEG = -1e30
W1 = 512             # sl1 band table width
W2 = 640             # sl2 band table width

AL = mybir.AluOpType
AF = mybir.ActivationFunctionType

_BIAS_SECTS = [
    ("q1", 8), ("q2", 8), ("q3", 8), ("o1", 8), ("o2", 8), ("o3", 8),
    ("f1", 32), ("f2", 8),
    ("ln1g", 8), ("ln1b", 8), ("ln2g", 8), ("ln2b", 8), ("ln3g", 8), ("ln3b", 8),
]
BIAS_COL = {}
_c = 0
for _n, _w in _BIAS_SECTS:
    BIAS_COL[_n] = _c
    _c += _w
NBIAS = _c  # 136


def _spans(sl):
    """Key-tile span [k0, k1] covered by the (h, l) band strip."""
    out = {}
    for l in range(4):
        if sl == 1:
            out[l] = (max(2 * l - 1, 0), 2 * l + 1)
        else:
            out[l] = (max(2 * l - 1, 0), min(2 * l + 2, 7))
    return out


def _ktplan(sl):
    """Per key-tile: (kt, col0, far0, blocks).  Parity-free."""
    spans = _spans(sl) if sl <= 2 else {}
    plan = []
    for kt in range(8):
        if sl == 1:
            c0, far0 = 128 * (kt // 2), None
        elif sl == 2:
            f = (kt + 3) // 2
            c0, far0 = 0, (128 * f if f < 4 else None)
        else:
            c0, far0 = 0, None
        blocks = []
        if sl <= 2:
            for l in range(4):
                k0, k1 = spans[l]
                if k0 <= kt <= k1:
                    blocks.append((l, (kt - k0) * 128))
        plan.append((kt, c0, far0, blocks))
    return plan


def _emit(nc, tc, ctx, T):
    te, ve, sc, gp, sy = nc.tensor, nc.vector, nc.scalar, nc.gpsimd, nc.sync

    singles = ctx.enter_context(tc.tile_pool(name="singles", bufs=1))
    psum = ctx.enter_context(tc.tile_pool(name="psum", bufs=1, space="PSUM"))
    wpool = ctx.enter_context(tc.tile_pool(name="wpool", bufs=2))
    work = ctx.enter_context(tc.tile_pool(name="work", bufs=2))
    smalls = ctx.enter_context(tc.tile_pool(name="smalls", bufs=2))
    dram = ctx.enter_context(tc.tile_pool(name="dramp", bufs=4, space="DRAM"))
    # PSUM budget (8 banks): s=4, pv=2, mm=2

    # ---- persistent SBUF ----
    bias_sb = singles.tile([128, NBIAS], F32)
    sy.dma_start(bias_sb, T["bias"])
    et1_sb = singles.tile([65, W1], BF)
    sy.dma_start(et1_sb, T["et1"])
    et2_sb = singles.tile([65, W2], BF)
    sy.dma_start(et2_sb, T["et2"])
    cv2_sb = singles.tile([64, 1], BF)
    sy.dma_start(cv2_sb, T["cv2"])
    g3t_sb = singles.tile([128, LK], BF)
    sy.dma_start(g3t_sb, T["g3t"])
    eps_sb = singles.tile([1, 1], F32)
    ve.memset(eps_sb, EPS)
    ones_sb = singles.tile([128, 1], BF)
    ve.memset(ones_sb, 1.0)
    ident = singles.tile([128, 128], BF)
    make_identity(nc, ident)

    xres = singles.tile([128, 8, S], F32)
    sy.dma_start(xres, T["xow"].rearrange("(a p) r -> p a r", p=128))
    xbf = singles.tile([128, 8, S], BF)
    sy.dma_start(xbf, T["xob"].rearrange("(a p) r -> p a r", p=128))

    big = singles.tile([128, 32, 512], BF)     # enc (slots 0..15) / FFN hidden
    kt_all = singles.tile([65, H, LK], BF)     # K^T per head + ones row
    gp.memset(kt_all[64:65, :, :], 1.0)
    v_all = singles.tile([128, H, 8, 65], BF)  # V key-major + ones col
    gp.memset(v_all, 1.0)
    qt = [singles.tile([65, S], BF, name=f"qt{h}") for h in range(H)]
    for h in range(H):
        gp.memset(qt[h][64:65, :], 1.0)
    aT = singles.tile([128, 8, S], BF)         # attention out (feature-major)

    def bias_ap(col, base=0, size=128):
        return bias_sb[base:base + size, col:col + 1]

    def load_enc(name):
        src = bass.AP(tensor=T[name].tensor, offset=T[name].offset,
                      ap=[[LK, 128], [LK * 128, 8], [512, 2], [1, 512]])
        dst = big[:, 0:16, :].rearrange("p (a b) r -> p a b r", a=8)
        sy.dma_start(dst, src)

    # ---------------- projections ----------------
    def proj_q(wname, bias0):
        wd = T[wname]
        for ct in range(8):
            wt = wpool.tile([128, 8, 128], BF, tag="w8", bufs=2,
                            name=f"w_{wname}_{ct}")
            sy.dma_start(wt, wd[ct].rearrange("a p r -> p a r"))
            ps = psum.tile([128, 512], F32, tag="mm", bufs=2,
                           name=f"qps_{wname}_{ct}")
            for f in range(8):
                te.matmul(ps, lhsT=wt[:, f, :], rhs=xbf[:, f, :],
                          start=(f == 0), stop=(f == 7))
            for sub in range(2):
                h = 2 * ct + sub
                ve.tensor_scalar_add(qt[h][0:64, :], ps[64 * sub:64 * sub + 64, :],
                                     bias_ap(bias0 + ct, 64 * sub, 64))

    def proj_k(wname, add_g):
        wd = T[wname]
        for ct in range(8):
            wt = wpool.tile([128, 8, 128], BF, tag="w8", bufs=2,
                            name=f"w_{wname}_{ct}")
            sy.dma_start(wt, wd[ct].rearrange("a p r -> p a r"))
            for rc in range(2):
                ps = psum.tile([128, 512], F32, tag="mm", bufs=2,
                               name=f"kps_{wname}_{ct}_{rc}")
                for f in range(8):
                    te.matmul(ps, lhsT=wt[:, f, :], rhs=big[:, 2 * f + rc, :],
                              start=(f == 0), stop=(f == 7))
                for sub in range(2):
                    h = 2 * ct + sub
                    dst = kt_all[0:64, h, 512 * rc:512 * rc + 512]
                    src = ps[64 * sub:64 * sub + 64, :]
                    if add_g:
                        ve.tensor_add(dst, src,
                                      g3t_sb[64 * sub:64 * sub + 64,
                                             512 * rc:512 * rc + 512])
                    else:
                        ve.tensor_copy(dst, src)

    def proj_v(wname):
        wd = T[wname]
        wvt = []
        for f in range(8):
            wt = wpool.tile([128, 1024], BF, tag="wv", bufs=8,
                            name=f"w_{wname}_{f}")
            sy.dma_start(wt, wd[f])
            wvt.append(wt)
        for kt in range(8):
            for half in range(2):
                ps = psum.tile([128, 512], F32, tag="mm", bufs=2,
                               name=f"vps_{wname}_{kt}_{half}")
                for f in range(8):
                    lhs = big[:, 2 * f + kt // 4,
                              128 * (kt % 4):128 * (kt % 4) + 128]
                    te.matmul(ps, lhsT=lhs,
                              rhs=wvt[f][:, 512 * half:512 * half + 512],
                              start=(f == 0), stop=(f == 7))
                dst = v_all[:, 8 * half:8 * half + 8, kt, 0:64]
                ve.tensor_copy(dst, ps.rearrange("p (a r) -> p a r", a=8))

    def cprime():
        for h in range(H):
            cps = psum.tile([1, 512], F32, tag="pv", bufs=2, name=f"cp_{h}")
            te.matmul(cps, lhsT=cv2_sb, rhs=qt[h][0:64, :], start=True, stop=True)
            ve.tensor_copy(qt[h][64:65, :], cps)

    # ---------------- bands ----------------
    def bands(sl):
        Wt = W1 if sl == 1 else W2
        et = et1_sb if sl == 1 else et2_sb
        bds = []
        for h in range(H):
            bd = dram.tile([128, 4 * Wt], BF, tag=f"bd{sl}", bufs=16,
                           name=f"bd{sl}_{h}")
            for l in range(4):
                for (c0, c1) in ([(0, Wt)] if Wt <= 512 else [(0, 512), (512, Wt)]):
                    bps = psum.tile([128, c1 - c0], F32, tag="mm", bufs=2,
                                    name=f"bps{sl}_{h}_{l}_{c0}")
                    te.matmul(bps, lhsT=qt[h][0:65, 128 * l:128 * l + 128],
                              rhs=et[0:65, c0:c1], start=True, stop=True)
                    bsb = work.tile([128, 512], BF, tag="bsb", bufs=3,
                                    name=f"bsb{sl}_{h}_{l}_{c0}")
                    ve.tensor_copy(bsb[:, 0:c1 - c0], bps)
                    sc.dma_start(bd[:, l * Wt + c0:l * Wt + c1],
                                 bsb[:, 0:c1 - c0])
            bds.append(bd)
        return bds

    # ---------------- attention ----------------
    def attention(sl, bds):
        plan = _ktplan(sl)
        spans = _spans(sl) if sl <= 2 else {}

        def emit_strips(h):
            out = {}
            if sl == 3:
                return out
            Wt = W1 if sl == 1 else W2
            bd = bds[h]
            for l in range(4):
                k0, k1 = spans[l]
                nk = k1 - k0 + 1
                st = work.tile([128, 512], BF, tag="strip", bufs=6,
                               name=f"st{sl}_{h}_{l}")
                base = l * Wt + 255 + 128 * (k0 - 2 * l)
                src = bass.AP(tensor=bd.tensor, offset=bd.offset + base,
                              ap=[[4 * Wt - 1, 128], [1, 128 * nk]])
                sy.dma_start(st[:, 0:128 * nk], src)
                out[l] = st
            return out

        def do_pv(h, pexp):
            pv = psum.tile([65, 512], F32, tag="pv", bufs=2, name=f"pv{sl}_{h}")
            n = len(plan)
            for idx, (kt, c0, _, _) in enumerate(plan):
                te.matmul(pv[:, c0:], lhsT=v_all[:, h, kt, :],
                          rhs=pexp[:, kt, c0:],
                          start=(idx == 0), stop=(idx == n - 1))
            rz = smalls.tile([1, 512], F32, tag="rz", bufs=1, name=f"rz{sl}_{h}")
            ve.reciprocal(rz, pv[64:65, :])
            zb = smalls.tile([64, 512], F32, tag="zb", bufs=1, name=f"zb{sl}_{h}")
            gp.partition_broadcast(zb, rz)
            ve.tensor_mul(aT[64 * (h % 2):64 * (h % 2) + 64, h // 2, :],
                          pv[0:64, :], zb)

        strips_cur = emit_strips(0)
        prev = None
        for h in range(H):
            strips_nxt = emit_strips(h + 1) if h + 1 < H else None
            pexp = work.tile([128, 8, 512], BF, tag="pexp", bufs=2,
                             name=f"pexp{sl}_{h}")
            for (kt, c0, far0, blocks) in plan:
                ps = psum.tile([128, 512], F32, tag="s", bufs=4,
                               name=f"s{sl}_{h}_{kt}")
                nw = (2 if far0 is not None else 1) + len(blocks)
                wi = [0]

                def fl():
                    wi[0] += 1
                    return wi[0] == 1, wi[0] == nw

                kslc = kt_all[:, h, 128 * kt:128 * kt + 128]
                if far0 is None:
                    s_, e_ = fl()
                    te.matmul(ps[:, c0:], lhsT=kslc[0:64], rhs=qt[h][0:64, c0:],
                              start=s_, stop=e_)
                else:
                    s_, e_ = fl()
                    te.matmul(ps[:, 0:far0], lhsT=kslc[0:64],
                              rhs=qt[h][0:64, 0:far0], start=s_, stop=e_)
                    s_, e_ = fl()
                    te.matmul(ps[:, far0:], lhsT=kslc[0:65],
                              rhs=qt[h][0:65, far0:], start=s_, stop=e_)
                for (l, off) in blocks:
                    s_, e_ = fl()
                    te.matmul(ps[:, 128 * l:128 * l + 128],
                              lhsT=strips_cur[l][:, off:off + 128], rhs=ident,
                              start=s_, stop=e_)
                sc.activation(pexp[:, kt, c0:], ps[:, c0:], AF.Exp,
                              scale=1.0 / SCALE)
            if prev is not None:
                do_pv(*prev)
            prev = (h, pexp)
            strips_cur = strips_nxt
        do_pv(*prev)

    # ---------------- output proj + residual ----------------
    def o_proj(wname, bias0):
        wd = T[wname]
        for ct in range(8):
            wt = wpool.tile([128, 8, 128], BF, tag="w8", bufs=2,
                            name=f"w_{wname}_{ct}")
            sy.dma_start(wt, wd[ct].rearrange("a p r -> p a r"))
            ps = psum.tile([128, 512], F32, tag="mm", bufs=2,
                           name=f"ops_{wname}_{ct}")
            for kt in range(8):
                te.matmul(ps, lhsT=wt[:, kt, :], rhs=aT[:, kt, :],
                          start=(kt == 0), stop=(kt == 7))
            ve.scalar_tensor_tensor(xres[:, ct, :], ps, bias_ap(bias0 + ct),
                                    xres[:, ct, :], op0=AL.add, op1=AL.add)

    # ---------------- layernorm ----------------
    _lnc = [0]

    def layer_norm(gname, bname, final=False):
        i = _lnc[0]
        _lnc[0] += 1
        s1 = psum.tile([1, 512], F32, tag="pv", bufs=2, name=f"lns1_{i}")
        s2 = psum.tile([1, 512], F32, tag="pv", bufs=2, name=f"lns2_{i}")
        for dt in range(8):
            sc.activation(xbf[:, dt, :], xres[:, dt, :], AF.Copy)
            sq = work.tile([128, 512], BF, tag="sq", bufs=2, name=f"sq_{i}_{dt}")
            ve.tensor_mul(sq, xres[:, dt, :], xres[:, dt, :])
            te.matmul(s1, lhsT=ones_sb, rhs=xbf[:, dt, :],
                      start=(dt == 0), stop=(dt == 7))
            te.matmul(s2, lhsT=ones_sb, rhs=sq,
                      start=(dt == 0), stop=(dt == 7))
        mean = smalls.tile([1, 512], F32, tag="lnm", bufs=1, name=f"mean_{i}")
        ve.tensor_scalar_mul(mean, s1, 1.0 / D)
        rstd = smalls.tile([1, 512], F32, tag="lnr", bufs=1, name=f"rstd_{i}")
        ve.tensor_mul(rstd, mean, mean)
        ve.scalar_tensor_tensor(rstd, s2, 1.0 / D, rstd,
                                op0=AL.mult, op1=AL.subtract)
        sc.activation(rstd, rstd, AF.Sqrt, bias=eps_sb)
        ve.reciprocal(rstd, rstd)
        mb = smalls.tile([128, 512], F32, tag="lnmb", bufs=1, name=f"mb_{i}")
        gp.partition_broadcast(mb, mean)
        rb = smalls.tile([128, 512], F32, tag="lnrb", bufs=1, name=f"rb_{i}")
        gp.partition_broadcast(rb, rstd)
        gcol, bcol = BIAS_COL[gname], BIAS_COL[bname]
        for dt in range(8):
            xm = work.tile([128, 512], F32, tag="lnx", bufs=2, name=f"xm_{i}_{dt}")
            ve.tensor_sub(xm, xres[:, dt, :], mb)
            t = work.tile([128, 512], F32, tag="lnx", bufs=2, name=f"t_{i}_{dt}")
            ve.scalar_tensor_tensor(t, xm, bias_ap(gcol + dt), rb,
                                    op0=AL.mult, op1=AL.mult)
            if final:
                ot = work.tile([128, 512], F32, tag="lnx", bufs=2,
                               name=f"ot_{i}_{dt}")
                ve.tensor_scalar_add(ot, t, bias_ap(bcol + dt))
                sy.dma_start(T["yT"][128 * dt:128 * dt + 128, :], ot)
            else:
                ve.tensor_scalar_add(xres[:, dt, :], t, bias_ap(bcol + dt))
                sc.activation(xbf[:, dt, :], xres[:, dt, :], AF.Copy)

    # ---------------- FFN ----------------
    def ffn():
        for ct in range(32):
            wt = wpool.tile([128, 8, 128], BF, tag="w8", bufs=2,
                            name=f"w_wf1_{ct}")
            sy.dma_start(wt, T["wf1"][ct].rearrange("a p r -> p a r"))
            ps = psum.tile([128, 512], F32, tag="mm", bufs=2, name=f"f1ps_{ct}")
            for f in range(8):
                te.matmul(ps, lhsT=wt[:, f, :], rhs=xbf[:, f, :],
                          start=(f == 0), stop=(f == 7))
            sc.activation(big[:, ct, :], ps, AF.Relu,
                          bias=bias_ap(BIAS_COL["f1"] + ct))
        for ct in range(8):
            wts = []
            for cc in range(2):
                wt = wpool.tile([128, 16, 128], BF, tag="wf2", bufs=2,
                                name=f"w_wf2_{ct}_{cc}")
                src = bass.AP(tensor=T["wf2"].tensor,
                              offset=T["wf2"].offset + (ct * 32 + 16 * cc) * 128 * 128,
                              ap=[[128, 128], [128 * 128, 16], [1, 128]])
                sy.dma_start(wt, src)
                wts.append(wt)
            ps = psum.tile([128, 512], F32, tag="mm", bufs=2, name=f"f2ps_{ct}")
            for kt in range(32):
                te.matmul(ps, lhsT=wts[kt // 16][:, kt % 16, :], rhs=big[:, kt, :],
                          start=(kt == 0), stop=(kt == 31))
            ve.scalar_tensor_tensor(xres[:, ct, :], ps, bias_ap(BIAS_COL["f2"] + ct),
                                    xres[:, ct, :], op0=AL.add, op1=AL.add)

    # ================= sublayer 1 =================
    load_enc("sfb")
    proj_q("wq1", BIAS_COL["q1"])
    bds = bands(1)
    proj_k("wk1", False)
    proj_v("wv1")
    attention(1, bds)
    o_proj("wo1", BIAS_COL["o1"])
    layer_norm("ln1g", "ln1b")

    # ================= sublayer 2 =================
    load_enc("chb")
    proj_q("wq2", BIAS_COL["q2"])
    cprime()
    bds = bands(2)
    proj_k("wk2", False)
    proj_v("wv2")
    attention(2, bds)
    o_proj("wo2", BIAS_COL["o2"])
    layer_norm("ln2g", "ln2b")

    # ================= sublayer 3 =================
    load_enc("wdb")
    proj_q("wq3", BIAS_COL["q3"])
    proj_k("wk3", True)
    proj_v("wv3")
    attention(3, None)
    o_proj("wo3", BIAS_COL["o3"])
    layer_norm("ln3g", "ln3b")

    # ================= FFN =================
    ffn()
    layer_norm("ln3g", "ln3b", final=True)


def build_nc():
    nc = bacc.Bacc("TRN2", target_bir_lowering=False, debug=False)
    T = {}

    def din(name, shape, dt=BF):
        T[name] = nc.dram_tensor(name, shape, dt, kind="ExternalInput").ap()

    din("xow", [D, S], F32)
    din("xob", [D, S])
    din("sfb", [D, LK])
    din("chb", [D, LK])
    din("wdb", [D, LK])
    for w in ["wq1", "wk1", "wo1", "wq2", "wk2", "wo2", "wq3", "wk3", "wo3"]:
        din(w, [8, 8, 128, 128])
    for w in ["wv1", "wv2", "wv3"]:
        din(w, [8, 128, 1024])
    din("wf1", [32, 8, 128, 128])
    din("wf2", [8, 32, 128, 128])
    din("bias", [128, NBIAS], F32)
    din("et1", [65, W1])
    din("et2", [65, W2])
    din("cv2", [64, 1])
    din("g3t", [128, LK])
    T["yT"] = nc.dram_tensor("yT", [D, S], F32, kind="ExternalOutput").ap()

    from contextlib import ExitStack
    with tile.TileContext(nc) as tc:
        with ExitStack() as ctx:
            _emit(nc, tc, ctx, T)
    nc.compile()
    return nc


_NC = None


def _get_nc():
    global _NC
    if _NC is None:
        _NC = build_nc()
    return _NC


# ======================= host side =======================

def _own_rows(pi):
    return np.concatenate([np.arange(128 * (2 * l + pi), 128 * (2 * l + pi) + 128)
                           for l in range(4)])


def _tile_ct(w):
    """[K, N] -> [N//128, K//128, 128, 128] bf16 (ct, f, p, n)."""
    K, N = w.shape
    return np.ascontiguousarray(
        w.reshape(K // 128, 128, N // 128, 128).transpose(2, 0, 1, 3)
    ).astype(BF16)


def _stripe(v):
    n = v.shape[0] // 128
    return np.ascontiguousarray(v.reshape(n, 128).T).astype(np.float32)


def _build_et(sl, pos, pi):
    W = W1 if sl == 1 else W2
    w = np.arange(W)
    t = w - 255 - 128 * pi
    et = np.zeros((65, W), np.float32)
    et[0:64, :] = pos[np.clip(t, -M, M) + M].T
    if sl == 1:
        et[0:64, :] -= pos[0][:, None]
        et[64, :] = np.where(t > 0, NEG, 0.0)
    else:
        et[0:64, :] -= pos[2 * M][:, None]
    return et.astype(BF16)


def _qpos(sentence_lengths):
    s = np.asarray(sentence_lengths, np.int64)
    offsets = s - np.cumsum(s)
    B = int(s.sum())
    return np.repeat(offsets, s)[:B] + np.arange(B)


def _host_prep(inp):
    f32 = lambda k: np.asarray(inp[k], np.float32)

    qkv_w = f32("qkv_w").reshape(D, H, 3, DH)
    wq1 = np.ascontiguousarray(qkv_w[:, :, 0, :].reshape(D, D))
    wk1 = np.ascontiguousarray(qkv_w[:, :, 1, :].reshape(D, D))
    wv1 = np.ascontiguousarray(qkv_w[:, :, 2, :].reshape(D, D))
    qkv_b = f32("qkv_b").reshape(H, 3, DH)
    q1_b = qkv_b[:, 0].reshape(D)
    v1_b = qkv_b[:, 2].reshape(D)

    o1_b = f32("o1_b") + v1_b @ f32("o1_w")
    o2_b = f32("o2_b") + f32("v2_b") @ f32("o2_w")
    o3_b = f32("o3_b") + f32("v3_b") @ f32("o3_w")

    bias = np.zeros((128, NBIAS), np.float32)

    def put(name, vec):
        c = BIAS_COL[name]
        s = _stripe(np.asarray(vec, np.float32))
        bias[:, c:c + s.shape[1]] = s

    put("q1", q1_b)
    put("q2", f32("q2_b"))
    put("q3", f32("q3_b"))
    put("o1", o1_b)
    put("o2", o2_b)
    put("o3", o3_b)
    put("f1", f32("f1_b"))
    put("f2", f32("f2_b"))
    for n, k in [("ln1g", "ln1_g"), ("ln1b", "ln1_b"), ("ln2g", "ln2_g"),
                 ("ln2b", "ln2_b"), ("ln3g", "ln3_g"), ("ln3b", "ln3_b")]:
        put(n, inp[k])

    weights = {
        "wq1": _tile_ct(wq1), "wk1": _tile_ct(wk1),
        "wv1": np.ascontiguousarray(wv1.reshape(8, 128, 1024)).astype(BF16),
        "wo1": _tile_ct(f32("o1_w")),
        "wq2": _tile_ct(f32("q2_w")), "wk2": _tile_ct(f32("k2_w")),
        "wv2": np.ascontiguousarray(f32("v2_w").reshape(8, 128, 1024)).astype(BF16),
        "wo2": _tile_ct(f32("o2_w")),
        "wq3": _tile_ct(f32("q3_w")), "wk3": _tile_ct(f32("k3_w")),
        "wv3": np.ascontiguousarray(f32("v3_w").reshape(8, 128, 1024)).astype(BF16),
        "wo3": _tile_ct(f32("o3_w")),
        "wf1": _tile_ct(f32("f1_w")),
        "wf2": _tile_ct(f32("f2_w")),
        "bias": bias,
    }

    pos1 = f32("pos1")
    pos2 = f32("pos2")
    pos3 = f32("pos3")
    et1 = [_build_et(1, pos1, pi) for pi in range(2)]
    et2 = [_build_et(2, pos2, pi) for pi in range(2)]
    cv2 = np.ascontiguousarray((pos2[0] - pos2[2 * M])[:, None]).astype(BF16)

    qpos = _qpos(inp["sentence_lengths"])
    g3 = []
    for b in range(4):
        idx = np.clip(np.arange(LK) - int(qpos[b]), -M, M) + M
        g = pos3[idx].T.astype(BF16)
        g3.append(np.concatenate([g, g], axis=0))

    x = f32("self_input")
    ch = f32("char_enc")
    wd = f32("word_enc")

    in_maps = []
    for core in range(8):
        b, pi = core // 2, core % 2
        rows = _own_rows(pi)
        xT = np.ascontiguousarray(x[b].T)
        m = dict(weights)
        m["xow"] = np.ascontiguousarray(xT[:, rows])
        m["xob"] = m["xow"].astype(BF16)
        m["sfb"] = xT.astype(BF16)
        m["chb"] = np.ascontiguousarray(ch[b].T).astype(BF16)
        m["wdb"] = np.ascontiguousarray(wd[b].T).astype(BF16)
        m["et1"] = et1[pi]
        m["et2"] = et2[pi]
        m["cv2"] = cv2
        m["g3t"] = g3[b]
        in_maps.append(m)
    return in_maps


def _fast_path_ok(inp):
    lam = np.asarray(inp["look_ahead_mask"])
    B, Lq = 4, 1024
    if lam.shape != (1, 1, Lq, Lq):
        return False
    causal = np.triu(np.ones((Lq, Lq), bool), k=1)
    if not np.array_equal(lam[0, 0].astype(bool), causal):
        return False
    if np.asarray(inp["char_mask"]).any() or np.asarray(inp["word_mask"]).any():
        return False
    if np.asarray(inp["sentence_lengths"]).sum() != B:
        return False
    return True


def _numpy_reference(inp):
    """Pure-numpy fallback (slow but exact) for unexpected mask patterns."""
    f = lambda k: np.asarray(inp[k], np.float32)

    def ln(x, g, b):
        m = x.mean(-1, keepdims=True)
        v = ((x - m) ** 2).mean(-1, keepdims=True)
        return (x - m) / np.sqrt(v + EPS) * g + b

    def split_heads(x):
        B, Sq, _ = x.shape
        return x.reshape(B, Sq, H, DH).transpose(0, 2, 1, 3)

    def softmax(x):
        x = x - x.max(-1, keepdims=True)
        e = np.exp(x)
        return e / e.sum(-1, keepdims=True)

    def attn(Q, K, V, pl, mask):
        logits = (np.einsum('bhid,bhjd->bhij', Q, K) + pl) / SCALE
        logits = np.where(mask, -np.inf, logits)
        p = softmax(logits)
        out = np.einsum('bhij,bhjd->bhid', p, V)
        B, h, Sq, dh = out.shape
        return out.transpose(0, 2, 1, 3).reshape(B, Sq, h * dh)

    def char_pos(emb, lq, lk):
        idx = np.clip(np.arange(lk)[None, :] - np.arange(lq)[:, None], -M, M) + M
        return emb[idx]

    x0 = f("self_input")
    B, Lq, _ = x0.shape
    qkv = (x0 @ f("qkv_w") + f("qkv_b")).reshape(B, Lq, H, 3 * DH).transpose(0, 2, 1, 3)
    Q, K, V = np.split(qkv, 3, axis=-1)
    pl = np.einsum('bhid,ijd->bhij', Q, char_pos(f("pos1"), Lq, Lq))
    a = attn(Q, K, V, pl, np.asarray(inp["look_ahead_mask"])) @ f("o1_w") + f("o1_b")
    x = ln(a + x0, f("ln1_g"), f("ln1_b"))

    ce = f("char_enc")
    Q = split_heads(x @ f("q2_w") + f("q2_b"))
    K = split_heads(ce @ f("k2_w") + f("k2_b"))
    V = split_heads(ce @ f("v2_w") + f("v2_b"))
    pl = np.einsum('bhid,ijd->bhij', Q, char_pos(f("pos2"), Lq, ce.shape[1]))
    a = attn(Q, K, V, pl, np.asarray(inp["char_mask"])) @ f("o2_w") + f("o2_b")
    x = ln(a + x, f("ln2_g"), f("ln2_b"))

    we = f("word_enc")
    Q = split_heads(x @ f("q3_w") + f("q3_b"))
    K = split_heads(we @ f("k3_w") + f("k3_b"))
    V = split_heads(we @ f("v3_w") + f("v3_b"))
    qpos = _qpos(inp["sentence_lengths"])
    idx = np.clip(np.arange(we.shape[1])[None, :] - qpos[:, None], -M, M) + M
    pl = np.einsum('bhid,bjd->bhij', Q, f("pos3")[idx])
    a = attn(Q, K, V, pl, np.asarray(inp["word_mask"])) @ f("o3_w") + f("o3_b")
    x = ln(a + x, f("ln3_g"), f("ln3_b"))

    ffn = np.maximum(x @ f("f1_w") + f("f1_b"), 0.0) @ f("f2_w") + f("f2_b")
    return ln(ffn + x, f("ln3_g"), f("ln3_b"))


def kernel(**inputs) -> np.ndarray:
    if not _fast_path_ok(inputs):
        return _numpy_reference(inputs)
    nc = _get_nc()
    in_maps = _host_prep(inputs)
    res = run_bass_kernel_spmd(nc, in_maps, list(range(8)))
    y = np.empty((4, 1024, 1024), np.float32)
    for core in range(8):
        b, pi = core // 2, core % 2
        yT = res.results[core]["yT"]
        y[b, _own_rows(pi), :] = yT.T
    return y


# revision 6
# speedup vs baseline: 3.7575x; 1.0750x over previous
"""Trainium2 Bass kernel for nn_DecoderLayer_84404697301735 (v3).

3-sublayer decoder (self-attn w/ char rel-pos, cross-attn to char encoder
w/ rel-pos, cross-attn to word encoder w/ word-level pos) + FFN.

Sharding: 8 cores = 4 batch x 2 interleaved query-tile halves; each core
computes 512 query rows end-to-end; K/V duplicated within a batch pair.
No collectives.  Program is parity-uniform (SPMD); parity is folded into
the band-table column origin (host data).

Scores are computed TRANSPOSED ([keys, queries]) so exp'd probabilities
feed PV matmuls directly -- no P transposes.  Relative-position logits use
softmax invariance (subtract Q.E_base) so the band only touches +-1
diagonal key tiles; strips are skew-read from a DRAM round trip and
transpose-accumulated into the scores PSUM via identity matmuls.  V is
computed key-major directly (enc stationary).  K/V biases fold away.
Phase order overlaps next-sublayer K/V projections with layernorm and
prefetches enc during attention to keep the PE warm.
"""

import numpy as np
import ml_dtypes

import concourse.bass as bass
import concourse.tile as tile
from concourse import bacc, mybir
from concourse.bass_utils import run_bass_kernel_spmd
from concourse.masks import make_identity

BF16 = ml_dtypes.bfloat16
F32 = mybir.dt.float32
BF = mybir.dt.bfloat16

D = 1024
H = 16
DH = 64
S = 512              # own query rows per core
LK = 1024            # keys
M = 128              # pos clip radius
SCALE = 8.0
EPS = 1e-5
NEG = -1e30
W1 = 512             # sl1 band table width
W2 = 640             # sl2 band table width

AL = mybir.AluOpType
AF = mybir.ActivationFunctionType

_BIAS_SECTS = [
    ("q1", 8), ("q2", 8), ("q3", 8), ("o1", 8), ("o2", 8), ("o3", 8),
    ("f1", 32), ("f2", 8),
    ("ln1g", 8), ("ln1b", 8), ("ln2g", 8), ("ln2b", 8), ("ln3g", 8), ("ln3b", 8),
]
BIAS_COL = {}
_c = 0
for _n, _w in _BIAS_SECTS:
    BIAS_COL[_n] = _c
    _c += _w
NBIAS = _c  # 136


def _spans(sl):
    """Key-tile span [k0, k1] covered by the (h, l) band strip."""
    out = {}
    for l in range(4):
        if sl == 1:
            out[l] = (max(2 * l - 1, 0), 2 * l + 1)
        else:
            out[l] = (max(2 * l - 1, 0), min(2 * l + 2, 7))
    return out


def _ktplan(sl):
    """Per key-tile: (kt, col0, far0, blocks).  Parity-free."""
    spans = _spans(sl) if sl <= 2 else {}
    plan = []
    for kt in range(8):
        if sl == 1:
            c0, far0 = 128 * (kt // 2), None
        elif sl == 2:
            f = (kt + 3) // 2
            c0, far0 = 0, (128 * f if f < 4 else None)
        else:
            c0, far0 = 0, None
        blocks = []
        if sl <= 2:
            for l in range(4):
                k0, k1 = spans[l]
                if k0 <= kt <= k1:
                    blocks.append((l, (kt - k0) * 128))
        plan.append((kt, c0, far0, blocks))
    return plan


def _emit(nc, tc, ctx, T):
    te, ve, sc, gp, sy = nc.tensor, nc.vector, nc.scalar, nc.gpsimd, nc.sync

    singles = ctx.enter_context(tc.tile_pool(name="singles", bufs=1))
    psum = ctx.enter_context(tc.tile_pool(name="psum", bufs=1, space="PSUM"))
    wpool = ctx.enter_context(tc.tile_pool(name="wpool", bufs=2))
    work = ctx.enter_context(tc.tile_pool(name="work", bufs=2))
    smalls = ctx.enter_context(tc.tile_pool(name="smalls", bufs=2))
    dram = ctx.enter_context(tc.tile_pool(name="dramp", bufs=4, space="DRAM"))
    # PSUM budget (8 banks): mm=2, s=2x(2 banks), pv=2

    # ---- persistent SBUF ----
    bias_sb = singles.tile([128, NBIAS], F32)
    sy.dma_start(bias_sb, T["bias"])
    et1_sb = singles.tile([65, W1], BF)
    sy.dma_start(et1_sb, T["et1"])
    et2_sb = singles.tile([65, W2], BF)
    sy.dma_start(et2_sb, T["et2"])
    cv2_sb = singles.tile([64, 1], BF)
    sy.dma_start(cv2_sb, T["cv2"])
    g3t_sb = singles.tile([128, LK], BF)
    sy.dma_start(g3t_sb, T["g3t"])
    eps_sb = singles.tile([1, 1], F32)
    ve.memset(eps_sb, EPS)
    ones_sb = singles.tile([128, 1], BF)
    ve.memset(ones_sb, 1.0)
    ident = singles.tile([128, 128], BF)
    make_identity(nc, ident)

    xres = singles.tile([128, 8, S], F32)
    sy.dma_start(xres, T["xow"].rearrange("(a p) r -> p a r", p=128))
    xbf = singles.tile([128, 8, S], BF)
    sy.dma_start(xbf, T["xob"].rearrange("(a p) r -> p a r", p=128))

    big = singles.tile([128, 32, 512], BF)     # enc (slots 0..15) / FFN hidden
    kt_all = singles.tile([65, H, LK], BF)     # K^T per head + ones row
    gp.memset(kt_all[64:65, :, :], 1.0)
    v_all = singles.tile([128, H, 8, 65], BF)  # V key-major + ones col
    gp.memset(v_all, 1.0)
    qt = [singles.tile([65, S], BF, name=f"qt{h}") for h in range(H)]
    for h in range(H):
        gp.memset(qt[h][64:65, :], 1.0)
    aT = singles.tile([128, 8, S], BF)         # attention out (feature-major)

    def bias_ap(col, base=0, size=128):
        return bias_sb[base:base + size, col:col + 1]

    def load_enc(name):
        src = bass.AP(tensor=T[name].tensor, offset=T[name].offset,
                      ap=[[LK, 128], [LK * 128, 8], [512, 2], [1, 512]])
        dst = big[:, 0:16, :].rearrange("p (a b) r -> p a b r", a=8)
        sy.dma_start(dst, src)

    # ---------------- projections ----------------
    def proj_q_ct(wname, bias0, ct):
        wt = wpool.tile([128, 8, 128], BF, tag="w8", bufs=2,
                        name=f"w_{wname}_{ct}")
        sy.dma_start(wt, T[wname][ct].rearrange("a p r -> p a r"))
        ps = psum.tile([128, 512], F32, tag="mm", bufs=2,
                       name=f"qps_{wname}_{ct}")
        for f in range(8):
            te.matmul(ps, lhsT=wt[:, f, :], rhs=xbf[:, f, :],
                      start=(f == 0), stop=(f == 7))
        for sub in range(2):
            h = 2 * ct + sub
            ve.tensor_scalar_add(qt[h][0:64, :], ps[64 * sub:64 * sub + 64, :],
                                 bias_ap(bias0 + ct, 64 * sub, 64))

    def proj_k(wname, add_g):
        wd = T[wname]
        for ct in range(8):
            wt = wpool.tile([128, 8, 128], BF, tag="w8", bufs=2,
                            name=f"w_{wname}_{ct}")
            sy.dma_start(wt, wd[ct].rearrange("a p r -> p a r"))
            for rc in range(2):
                ps = psum.tile([128, 512], F32, tag="mm", bufs=2,
                               name=f"kps_{wname}_{ct}_{rc}")
                for f in range(8):
                    te.matmul(ps, lhsT=wt[:, f, :], rhs=big[:, 2 * f + rc, :],
                              start=(f == 0), stop=(f == 7))
                for sub in range(2):
                    h = 2 * ct + sub
                    dst = kt_all[0:64, h, 512 * rc:512 * rc + 512]
                    src = ps[64 * sub:64 * sub + 64, :]
                    if add_g:
                        ve.tensor_add(dst, src,
                                      g3t_sb[64 * sub:64 * sub + 64,
                                             512 * rc:512 * rc + 512])
                    else:
                        ve.tensor_copy(dst, src)

    def proj_v(wname):
        wd = T[wname]
        wvt = []
        for f in range(8):
            wt = wpool.tile([128, 1024], BF, tag="wv", bufs=8,
                            name=f"w_{wname}_{f}")
            sy.dma_start(wt, wd[f])
            wvt.append(wt)
        for kt in range(8):
            for half in range(2):
                ps = psum.tile([128, 512], F32, tag="mm", bufs=2,
                               name=f"vps_{wname}_{kt}_{half}")
                for f in range(8):
                    lhs = big[:, 2 * f + kt // 4,
                              128 * (kt % 4):128 * (kt % 4) + 128]
                    te.matmul(ps, lhsT=lhs,
                              rhs=wvt[f][:, 512 * half:512 * half + 512],
                              start=(f == 0), stop=(f == 7))
                dst = v_all[:, 8 * half:8 * half + 8, kt, 0:64]
                ve.tensor_copy(dst, ps.rearrange("p (a r) -> p a r", a=8))

    def cprime_h(h):
        cps = psum.tile([1, 512], F32, tag="pv", bufs=2, name=f"cp_{h}")
        te.matmul(cps, lhsT=cv2_sb, rhs=qt[h][0:64, :], start=True, stop=True)
        ve.tensor_copy(qt[h][64:65, :], cps)

    # ---------------- bands ----------------
    def bands_h(sl, h):
        Wt = W1 if sl == 1 else W2
        et = et1_sb if sl == 1 else et2_sb
        bd = dram.tile([128, 4 * Wt], BF, tag=f"bd{sl}", bufs=16,
                       name=f"bd{sl}_{h}")
        eng = h
        for l in range(4):
            for (c0, c1) in ([(0, Wt)] if Wt <= 512 else [(0, 512), (512, Wt)]):
                bps = psum.tile([128, c1 - c0], F32, tag="mm", bufs=2,
                                name=f"bps{sl}_{h}_{l}_{c0}")
                te.matmul(bps, lhsT=qt[h][0:65, 128 * l:128 * l + 128],
                          rhs=et[0:65, c0:c1], start=True, stop=True)
                bsb = work.tile([128, 512], BF, tag="bsb", bufs=4,
                                name=f"bsb{sl}_{h}_{l}_{c0}")
                if eng % 2 == 0:
                    ve.tensor_copy(bsb[:, 0:c1 - c0], bps)
                else:
                    sc.activation(bsb[:, 0:c1 - c0], bps, AF.Copy)
                eng += 1
                sc.dma_start(bd[:, l * Wt + c0:l * Wt + c1], bsb[:, 0:c1 - c0])
        return bd

    def q_and_bands(sl, wname, bias0):
        bds = []
        for ct in range(8):
            proj_q_ct(wname, bias0, ct)
            for sub in range(2):
                h = 2 * ct + sub
                if sl == 2:
                    cprime_h(h)
                bds.append(bands_h(sl, h))
        return bds

    # ---------------- attention ----------------
    def attention(sl, bds):
        plan = _ktplan(sl)
        spans = _spans(sl) if sl <= 2 else {}

        def emit_strips(h):
            out = {}
            if sl == 3:
                return out
            Wt = W1 if sl == 1 else W2
            bd = bds[h]
            for l in range(4):
                k0, k1 = spans[l]
                nk = k1 - k0 + 1
                st = work.tile([128, 512], BF, tag="strip", bufs=6,
                               name=f"st{sl}_{h}_{l}")
                base = l * Wt + 255 + 128 * (k0 - 2 * l)
                src = bass.AP(tensor=bd.tensor, offset=bd.offset + base,
                              ap=[[4 * Wt - 1, 128], [1, 128 * nk]])
                sy.dma_start(st[:, 0:128 * nk], src)
                out[l] = st
            return out

        def do_pv(h, pexp):
            pv = psum.tile([65, 512], F32, tag="pv", bufs=2, name=f"pv{sl}_{h}")
            n = len(plan)
            for idx, (kt, c0, _, _) in enumerate(plan):
                te.matmul(pv[:, c0:], lhsT=v_all[:, h, kt, :],
                          rhs=pexp[:, kt, c0:],
                          start=(idx == 0), stop=(idx == n - 1))
            rz = smalls.tile([1, 512], F32, tag="rz", bufs=2, name=f"rz{sl}_{h}")
            ve.reciprocal(rz, pv[64:65, :])
            zb = work.tile([64, 512], F32, tag="lnx", bufs=2, name=f"zb{sl}_{h}")
            gp.partition_broadcast(zb, rz)
            ve.tensor_mul(aT[64 * (h % 2):64 * (h % 2) + 64, h // 2, :],
                          pv[0:64, :], zb)

        strips_cur = emit_strips(0)
        prev = None
        for h in range(H):
            strips_nxt = emit_strips(h + 1) if h + 1 < H else None
            pexp = work.tile([128, 8, 512], BF, tag="pexp", bufs=2,
                             name=f"pexp{sl}_{h}")
            for p in range(4):
                sp = psum.tile([128, 1024], F32, tag="s", bufs=2,
                               name=f"s{sl}_{h}_{p}")
                spv = sp.rearrange("p (a r) -> p a r", a=2)
                for sub in range(2):
                    kt, c0, far0, blocks = plan[2 * p + sub]
                    off = 512 * sub
                    nw = (2 if far0 is not None else 1) + len(blocks)
                    wi = [0]

                    def fl():
                        wi[0] += 1
                        return wi[0] == 1, wi[0] == nw

                    kslc = kt_all[:, h, 128 * kt:128 * kt + 128]
                    if far0 is None:
                        s_, e_ = fl()
                        te.matmul(sp[:, off + c0:off + 512], lhsT=kslc[0:64],
                                  rhs=qt[h][0:64, c0:], start=s_, stop=e_)
                    else:
                        s_, e_ = fl()
                        te.matmul(sp[:, off:off + far0], lhsT=kslc[0:64],
                                  rhs=qt[h][0:64, 0:far0], start=s_, stop=e_)
                        s_, e_ = fl()
                        te.matmul(sp[:, off + far0:off + 512], lhsT=kslc[0:65],
                                  rhs=qt[h][0:65, far0:], start=s_, stop=e_)
                    for (l, soff) in blocks:
                        s_, e_ = fl()
                        te.matmul(sp[:, off + 128 * l:off + 128 * l + 128],
                                  lhsT=strips_cur[l][:, soff:soff + 128],
                                  rhs=ident, start=s_, stop=e_)
                c0 = plan[2 * p][1]
                sc.activation(pexp[:, 2 * p:2 * p + 2, c0:], spv[:, :, c0:],
                              AF.Exp, scale=1.0 / SCALE)
            if prev is not None:
                do_pv(*prev)
            prev = (h, pexp)
            strips_cur = strips_nxt
        do_pv(*prev)

    # ---------------- output proj + residual ----------------
    def o_proj(wname, bias0):
        wd = T[wname]
        for ct in range(8):
            wt = wpool.tile([128, 8, 128], BF, tag="w8", bufs=2,
                            name=f"w_{wname}_{ct}")
            sy.dma_start(wt, wd[ct].rearrange("a p r -> p a r"))
            ps = psum.tile([128, 512], F32, tag="mm", bufs=2,
                           name=f"ops_{wname}_{ct}")
            for kt in range(8):
                te.matmul(ps, lhsT=wt[:, kt, :], rhs=aT[:, kt, :],
                          start=(kt == 0), stop=(kt == 7))
            ve.scalar_tensor_tensor(xres[:, ct, :], ps, bias_ap(bias0 + ct),
                                    xres[:, ct, :], op0=AL.add, op1=AL.add)

    # ---------------- layernorm ----------------
    def ln_stats(i):
        s1 = psum.tile([1, 512], F32, tag="pv", bufs=2, name=f"lns1_{i}")
        s2 = psum.tile([1, 512], F32, tag="pv", bufs=2, name=f"lns2_{i}")
        for dt in range(8):
            sc.activation(xbf[:, dt, :], xres[:, dt, :], AF.Copy)
            sq = work.tile([128, 512], BF, tag="sq", bufs=2, name=f"sq_{i}_{dt}")
            ve.tensor_mul(sq, xres[:, dt, :], xres[:, dt, :])
            te.matmul(s1, lhsT=ones_sb, rhs=xbf[:, dt, :],
                      start=(dt == 0), stop=(dt == 7))
            te.matmul(s2, lhsT=ones_sb, rhs=sq,
                      start=(dt == 0), stop=(dt == 7))
        return s1, s2

    def ln_finish(i, s1, s2, gname, bname, final=False):
        mean = smalls.tile([1, 512], F32, tag="lnm", bufs=1, name=f"mean_{i}")
        ve.tensor_scalar_mul(mean, s1, 1.0 / D)
        rstd = smalls.tile([1, 512], F32, tag="lnr", bufs=1, name=f"rstd_{i}")
        ve.tensor_mul(rstd, mean, mean)
        ve.scalar_tensor_tensor(rstd, s2, 1.0 / D, rstd,
                                op0=AL.mult, op1=AL.subtract)
        sc.activation(rstd, rstd, AF.Sqrt, bias=eps_sb)
        ve.reciprocal(rstd, rstd)
        mb = smalls.tile([128, 512], F32, tag="lnmb", bufs=1, name=f"mb_{i}")
        gp.partition_broadcast(mb, mean)
        rb = smalls.tile([128, 512], F32, tag="lnrb", bufs=1, name=f"rb_{i}")
        gp.partition_broadcast(rb, rstd)
        gcol, bcol = BIAS_COL[gname], BIAS_COL[bname]
        for dt in range(8):
            xm = work.tile([128, 512], F32, tag="lnx", bufs=2, name=f"xm_{i}_{dt}")
            ve.tensor_sub(xm, xres[:, dt, :], mb)
            t = work.tile([128, 512], F32, tag="lnx", bufs=2, name=f"t_{i}_{dt}")
            ve.scalar_tensor_tensor(t, xm, bias_ap(gcol + dt), rb,
                                    op0=AL.mult, op1=AL.mult)
            if final:
                ot = work.tile([128, 512], F32, tag="lnx", bufs=2,
                               name=f"ot_{i}_{dt}")
                ve.tensor_scalar_add(ot, t, bias_ap(bcol + dt))
                sy.dma_start(T["yT"][128 * dt:128 * dt + 128, :], ot)
            else:
                ve.tensor_scalar_add(xres[:, dt, :], t, bias_ap(bcol + dt))
                sc.activation(xbf[:, dt, :], xres[:, dt, :], AF.Copy)

    # ---------------- FFN ----------------
    def ffn():
        for ct in range(32):
            wt = wpool.tile([128, 8, 128], BF, tag="w8", bufs=2,
                            name=f"w_wf1_{ct}")
            sy.dma_start(wt, T["wf1"][ct].rearrange("a p r -> p a r"))
            ps = psum.tile([128, 512], F32, tag="mm", bufs=2, name=f"f1ps_{ct}")
            for f in range(8):
                te.matmul(ps, lhsT=wt[:, f, :], rhs=xbf[:, f, :],
                          start=(f == 0), stop=(f == 7))
            sc.activation(big[:, ct, :], ps, AF.Relu,
                          bias=bias_ap(BIAS_COL["f1"] + ct))
        for ct in range(8):
            wts = []
            for cc in range(2):
                wt = wpool.tile([128, 16, 128], BF, tag="wf2", bufs=2,
                                name=f"w_wf2_{ct}_{cc}")
                src = bass.AP(tensor=T["wf2"].tensor,
                              offset=T["wf2"].offset + (ct * 32 + 16 * cc) * 128 * 128,
                              ap=[[128, 128], [128 * 128, 16], [1, 128]])
                sy.dma_start(wt, src)
                wts.append(wt)
            ps = psum.tile([128, 512], F32, tag="mm", bufs=2, name=f"f2ps_{ct}")
            for kt in range(32):
                te.matmul(ps, lhsT=wts[kt // 16][:, kt % 16, :], rhs=big[:, kt, :],
                          start=(kt == 0), stop=(kt == 31))
            ve.scalar_tensor_tensor(xres[:, ct, :], ps, bias_ap(BIAS_COL["f2"] + ct),
                                    xres[:, ct, :], op0=AL.add, op1=AL.add)

    # ================= schedule =================
    load_enc("sfb")
    with nc.named_scope("s1_qb"):
        bds = q_and_bands(1, "wq1", BIAS_COL["q1"])
    with nc.named_scope("s1_kv"):
        proj_k("wk1", False)
        proj_v("wv1")
    load_enc("chb")
    with nc.named_scope("s1_attn"):
        attention(1, bds)
    with nc.named_scope("s1_oln"):
        o_proj("wo1", BIAS_COL["o1"])
        st = ln_stats(0)
    with nc.named_scope("s2_kv"):
        proj_k("wk2", False)
        proj_v("wv2")
    with nc.named_scope("s1_lnf"):
        ln_finish(0, *st, "ln1g", "ln1b")
    with nc.named_scope("s2_qb"):
        bds = q_and_bands(2, "wq2", BIAS_COL["q2"])
    load_enc("wdb")
    with nc.named_scope("s2_attn"):
        attention(2, bds)
    with nc.named_scope("s2_oln"):
        o_proj("wo2", BIAS_COL["o2"])
        st = ln_stats(1)
    with nc.named_scope("s3_kv"):
        proj_k("wk3", True)
        proj_v("wv3")
    with nc.named_scope("s2_lnf"):
        ln_finish(1, *st, "ln2g", "ln2b")
    with nc.named_scope("s3_q"):
        for ct in range(8):
            proj_q_ct("wq3", BIAS_COL["q3"], ct)
    with nc.named_scope("s3_attn"):
        attention(3, None)
    with nc.named_scope("s3_oln"):
        o_proj("wo3", BIAS_COL["o3"])
        st = ln_stats(2)
        ln_finish(2, *st, "ln3g", "ln3b")
    with nc.named_scope("ffn"):
        ffn()
    with nc.named_scope("lnf"):
        st = ln_stats(3)
        ln_finish(3, *st, "ln3g", "ln3b", final=True)


def build_nc():
    nc = bacc.Bacc("TRN2", target_bir_lowering=False, debug=False)
    T = {}

    def din(name, shape, dt=BF):
        T[name] = nc.dram_tensor(name, shape, dt, kind="ExternalInput").ap()

    din("xow", [D, S], F32)
    din("xob", [D, S])
    din("sfb", [D, LK])
    din("chb", [D, LK])
    din("wdb", [D, LK])
    for w in ["wq1", "wk1", "wo1", "wq2", "wk2", "wo2", "wq3", "wk3", "wo3"]:
        din(w, [8, 8, 128, 128])
    for w in ["wv1", "wv2", "wv3"]:
        din(w, [8, 128, 1024])
    din("wf1", [32, 8, 128, 128])
    din("wf2", [8, 32, 128, 128])
    din("bias", [128, NBIAS], F32)
    din("et1", [65, W1])
    din("et2", [65, W2])
    din("cv2", [64, 1])
    din("g3t", [128, LK])
    T["yT"] = nc.dram_tensor("yT", [D, S], F32, kind="ExternalOutput").ap()

    from contextlib import ExitStack
    with tile.TileContext(nc) as tc:
        with ExitStack() as ctx:
            _emit(nc, tc, ctx, T)
    nc.compile()
    return nc


_NC = None


def _get_nc():
    global _NC
    if _NC is None:
        _NC = build_nc()
    return _NC


# ======================= host side =======================

def _own_rows(pi):
    return np.concatenate([np.arange(128 * (2 * l + pi), 128 * (2 * l + pi) + 128)
                           for l in range(4)])


def _tile_ct(w):
    """[K, N] -> [N//128, K//128, 128, 128] bf16 (ct, f, p, n)."""
    K, N = w.shape
    return np.ascontiguousarray(
        w.reshape(K // 128, 128, N // 128, 128).transpose(2, 0, 1, 3)
    ).astype(BF16)


def _stripe(v):
    n = v.shape[0] // 128
    return np.ascontiguousarray(v.reshape(n, 128).T).astype(np.float32)


def _build_et(sl, pos, pi):
    W = W1 if sl == 1 else W2
    w = np.arange(W)
    t = w - 255 - 128 * pi
    et = np.zeros((65, W), np.float32)
    et[0:64, :] = pos[np.clip(t, -M, M) + M].T
    if sl == 1:
        et[0:64, :] -= pos[0][:, None]
        et[64, :] = np.where(t > 0, NEG, 0.0)
    else:
        et[0:64, :] -= pos[2 * M][:, None]
    return et.astype(BF16)


def _qpos(sentence_lengths):
    s = np.asarray(sentence_lengths, np.int64)
    offsets = s - np.cumsum(s)
    B = int(s.sum())
    return np.repeat(offsets, s)[:B] + np.arange(B)


def _host_prep(inp):
    f32 = lambda k: np.asarray(inp[k], np.float32)

    qkv_w = f32("qkv_w").reshape(D, H, 3, DH)
    wq1 = np.ascontiguousarray(qkv_w[:, :, 0, :].reshape(D, D))
    wk1 = np.ascontiguousarray(qkv_w[:, :, 1, :].reshape(D, D))
    wv1 = np.ascontiguousarray(qkv_w[:, :, 2, :].reshape(D, D))
    qkv_b = f32("qkv_b").reshape(H, 3, DH)
    q1_b = qkv_b[:, 0].reshape(D)
    v1_b = qkv_b[:, 2].reshape(D)

    o1_b = f32("o1_b") + v1_b @ f32("o1_w")
    o2_b = f32("o2_b") + f32("v2_b") @ f32("o2_w")
    o3_b = f32("o3_b") + f32("v3_b") @ f32("o3_w")

    bias = np.zeros((128, NBIAS), np.float32)

    def put(name, vec):
        c = BIAS_COL[name]
        s = _stripe(np.asarray(vec, np.float32))
        bias[:, c:c + s.shape[1]] = s

    put("q1", q1_b)
    put("q2", f32("q2_b"))
    put("q3", f32("q3_b"))
    put("o1", o1_b)
    put("o2", o2_b)
    put("o3", o3_b)
    put("f1", f32("f1_b"))
    put("f2", f32("f2_b"))
    for n, k in [("ln1g", "ln1_g"), ("ln1b", "ln1_b"), ("ln2g", "ln2_g"),
                 ("ln2b", "ln2_b"), ("ln3g", "ln3_g"), ("ln3b", "ln3_b")]:
        put(n, inp[k])

    weights = {
        "wq1": _tile_ct(wq1), "wk1": _tile_ct(wk1),
        "wv1": np.ascontiguousarray(wv1.reshape(8, 128, 1024)).astype(BF16),
        "wo1": _tile_ct(f32("o1_w")),
        "wq2": _tile_ct(f32("q2_w")), "wk2": _tile_ct(f32("k2_w")),
        "wv2": np.ascontiguousarray(f32("v2_w").reshape(8, 128, 1024)).astype(BF16),
        "wo2": _tile_ct(f32("o2_w")),
        "wq3": _tile_ct(f32("q3_w")), "wk3": _tile_ct(f32("k3_w")),
        "wv3": np.ascontiguousarray(f32("v3_w").reshape(8, 128, 1024)).astype(BF16),
        "wo3": _tile_ct(f32("o3_w")),
        "wf1": _tile_ct(f32("f1_w")),
        "wf2": _tile_ct(f32("f2_w")),
        "bias": bias,
    }

    pos1 = f32("pos1")
    pos2 = f32("pos2")
    pos3 = f32("pos3")
    et1 = [_build_et(1, pos1, pi) for pi in range(2)]
    et2 = [_build_et(2, pos2, pi) for pi in range(2)]
    cv2 = np.ascontiguousarray((pos2[0] - pos2[2 * M])[:, None]).astype(BF16)

    qpos = _qpos(inp["sentence_lengths"])
    g3 = []
    for b in range(4):
        idx = np.clip(np.arange(LK) - int(qpos[b]), -M, M) + M
        g = pos3[idx].T.astype(BF16)
        g3.append(np.concatenate([g, g], axis=0))

    x = f32("self_input")
    ch = f32("char_enc")
    wd = f32("word_enc")

    in_maps = []
    for core in range(8):
        b, pi = core // 2, core % 2
        rows = _own_rows(pi)
        xT = np.ascontiguousarray(x[b].T)
        m = dict(weights)
        m["xow"] = np.ascontiguousarray(xT[:, rows])
        m["xob"] = m["xow"].astype(BF16)
        m["sfb"] = xT.astype(BF16)
        m["chb"] = np.ascontiguousarray(ch[b].T).astype(BF16)
        m["wdb"] = np.ascontiguousarray(wd[b].T).astype(BF16)
        m["et1"] = et1[pi]
        m["et2"] = et2[pi]
        m["cv2"] = cv2
        m["g3t"] = g3[b]
        in_maps.append(m)
    return in_maps


def _fast_path_ok(inp):
    lam = np.asarray(inp["look_ahead_mask"])
    B, Lq = 4, 1024
    if lam.shape != (1, 1, Lq, Lq):
        return False
    causal = np.triu(np.ones((Lq, Lq), bool), k=1)
    if not np.array_equal(lam[0, 0].astype(bool), causal):
        return False
    if np.asarray(inp["char_mask"]).any() or np.asarray(inp["word_mask"]).any():
        return False
    if np.asarray(inp["sentence_lengths"]).sum() != B:
        return False
    return True


def _numpy_reference(inp):
    """Pure-numpy fallback (slow but exact) for unexpected mask patterns."""
    f = lambda k: np.asarray(inp[k], np.float32)

    def ln(x, g, b):
        m = x.mean(-1, keepdims=True)
        v = ((x - m) ** 2).mean(-1, keepdims=True)
        return (x - m) / np.sqrt(v + EPS) * g + b

    def split_heads(x):
        B, Sq, _ = x.shape
        return x.reshape(B, Sq, H, DH).transpose(0, 2, 1, 3)

    def softmax(x):
        x = x - x.max(-1, keepdims=True)
        e = np.exp(x)
        return e / e.sum(-1, keepdims=True)

    def attn(Q, K, V, pl, mask):
        logits = (np.einsum('bhid,bhjd->bhij', Q, K) + pl) / SCALE
        logits = np.where(mask, -np.inf, logits)
        p = softmax(logits)
        out = np.einsum('bhij,bhjd->bhid', p, V)
        B, h, Sq, dh = out.shape
        return out.transpose(0, 2, 1, 3).reshape(B, Sq, h * dh)

    def char_pos(emb, lq, lk):
        idx = np.clip(np.arange(lk)[None, :] - np.arange(lq)[:, None], -M, M) + M
        return emb[idx]

    x0 = f("self_input")
    B, Lq, _ = x0.shape
    qkv = (x0 @ f("qkv_w") + f("qkv_b")).reshape(B, Lq, H, 3 * DH).transpose(0, 2, 1, 3)
    Q, K, V = np.split(qkv, 3, axis=-1)
    pl = np.einsum('bhid,ijd->bhij', Q, char_pos(f("pos1"), Lq, Lq))
    a = attn(Q, K, V, pl, np.asarray(inp["look_ahead_mask"])) @ f("o1_w") + f("o1_b")
    x = ln(a + x0, f("ln1_g"), f("ln1_b"))

    ce = f("char_enc")
    Q = split_heads(x @ f("q2_w") + f("q2_b"))
    K = split_heads(ce @ f("k2_w") + f("k2_b"))
    V = split_heads(ce @ f("v2_w") + f("v2_b"))
    pl = np.einsum('bhid,ijd->bhij', Q, char_pos(f("pos2"), Lq, ce.shape[1]))
    a = attn(Q, K, V, pl, np.asarray(inp["char_mask"])) @ f("o2_w") + f("o2_b")
    x = ln(a + x, f("ln2_g"), f("ln2_b"))

    we = f("word_enc")
    Q = split_heads(x @ f("q3_w") + f("q3_b"))
    K = split_heads(we @ f("k3_w") + f("k3_b"))
    V = split_heads(we @ f("v3_w") + f("v3_b"))
    qpos = _qpos(inp["sentence_lengths"])
    idx = np.clip(np.arange(we.shape[1])[None, :] - qpos[:, None], -M, M) + M
    pl = np.einsum('bhid,bjd->bhij', Q, f("pos3")[idx])
    a = attn(Q, K, V, pl, np.asarray(inp["word_mask"])) @ f("o3_w") + f("o3_b")
    x = ln(a + x, f("ln3_g"), f("ln3_b"))

    ffn = np.maximum(x @ f("f1_w") + f("f1_b"), 0.0) @ f("f2_w") + f("f2_b")
    return ln(ffn + x, f("ln3_g"), f("ln3_b"))


def kernel(**inputs) -> np.ndarray:
    if not _fast_path_ok(inputs):
        return _numpy_reference(inputs)
    nc = _get_nc()
    in_maps = _host_prep(inputs)
    res = run_bass_kernel_spmd(nc, in_maps, list(range(8)))
    y = np.empty((4, 1024, 1024), np.float32)
    for core in range(8):
        b, pi = core // 2, core % 2
        yT = res.results[core]["yT"]
        y[b, _own_rows(pi), :] = yT.T
    return y
